# revision 27
# baseline (speedup 1.0000x reference)
"""Efficient Channel Attention kernel for 8 Trainium2 NeuronCores.

Problem (B=4, N=4096, C=1024, H=4, HD=256):
    qkv = x @ Wqkv.T; q,k l2-normalized over N; scores = (q*temp) @ k.T
    attn = softmax(scores, -1); out = attn @ v; y = out @ Wproj.T + bproj + x

Sharding: core = (batch b, head-pair hp). Each core computes heads
{2hp, 2hp+1} over ALL 4096 tokens of its batch, so the token-contracted
Grams and q/k norms are complete locally — NO collective at all. The
output rows owned by head h are y[h*1024:(h+1)*1024], so each core owns
the contiguous y rows [2048*hp, 2048*hp+2048).

Token permutation: on-chip local token index l = m*1024 + j' maps to
global token n = 4*j' + m (the torch transpose+reshape scramble). With
this order v/outT are naturally m-blocked and every matmul operand is
contiguous (no strided rhs — 8x penalty measured on HW).

Dtypes: the big GEMMs (qkv-projection, v-projection, proj) and the
Grams run in fp8e4m3 with DoubleRow perf mode (K=256/pass, 2x
throughput); weights host-scaled x16 so fp8 operands sit in the normal
range; the 1/256 net scale folds into the proj output activation. The
q/k sumsq for l2-norm comes from PE self-Gram diagonal blocks (no
vector-engine square/accumulate chains). attn@v runs in bf16;
norms/softmax/residual in fp32.
"""

import numpy as np

B, N, C, H = 4, 4096, 1024, 4
HD = C // H          # 256
NCORES = 8
NT = 32              # 128-token sub-tiles
SW = 16.0            # host weight scale for fp8 range
EPS = 1e-12

_CACHE = {}


def _build():
    import concourse.mybir as mybir
    import concourse.tile as tile
    from concourse import bacc
    from concourse.masks import make_identity

    f32 = mybir.dt.float32
    bf16 = mybir.dt.bfloat16
    fp8 = mybir.dt.float8e4
    DR = mybir.MatmulPerfMode.DoubleRow
    AX = mybir.AxisListType.X
    Exp = mybir.ActivationFunctionType.Exp
    Ident = mybir.ActivationFunctionType.Identity

    nc = bacc.Bacc("TRN2", target_bir_lowering=False, debug=False,
                   num_devices=NCORES)

    # paired-ktile layouts: row 128*j+p, free (i, n) holds src[256j+128i+p, n]
    xT_d = nc.dram_tensor("xT", [512, 2 * N], fp8, kind="ExternalInput").ap()
    wqk_d = nc.dram_tensor("wqk", [512, 2048], fp8, kind="ExternalInput").ap()
    wv_d = nc.dram_tensor("wv", [512, 1024], fp8, kind="ExternalInput").ap()
    wp_d = nc.dram_tensor("wp", [512, 2048], fp8, kind="ExternalInput").ap()
    xrT_d = nc.dram_tensor("xrT", [C, 2048], f32, kind="ExternalInput").ap()
    bias_d = nc.dram_tensor("bias", [128, 8], f32, kind="ExternalInput").ap()
    tmpv_d = nc.dram_tensor("tmpv", [128, 4], f32, kind="ExternalInput").ap()
    yT_d = nc.dram_tensor("yT", [C, 2048], f32, kind="ExternalOutput").ap()

    with tile.TileContext(nc) as tc:
        with (
            tc.tile_pool(name="const", bufs=1) as constp,
            tc.tile_pool(name="wgt", bufs=1) as wgtp,
            tc.tile_pool(name="xs", bufs=1) as xsp,
            tc.tile_pool(name="vo", bufs=1) as vop,
            tc.tile_pool(name="wrk", bufs=1) as wrk,
            tc.tile_pool(name="ps", bufs=1, space="PSUM") as ps,
        ):
            # ---------------- constants + weights ----------------
            ident = constp.tile([128, 128], f32, name="ident")
            make_identity(nc, ident[:])
            # 4 identity blocks side by side, for diag extraction
            identx = constp.tile([128, 512], f32, name="identx")
            for i in range(4):
                make_identity(nc, identx[:, i * 128:(i + 1) * 128])
            bias_sb = constp.tile([128, 8], f32, name="bias_sb")
            nc.sync.dma_start(bias_sb[:], bias_d[:])
            tmpv_sb = constp.tile([128, 4], f32, name="tmpv_sb")
            nc.sync.dma_start(tmpv_sb[:], tmpv_d[:])

            # qk weights first (the first matmul group needs all of wqk),
            # then x quarter 0, then the rest interleaved by need-time.
            wqk = [wgtp.tile([128, 2, 1024], fp8, name=f"wqk{jp}")
                   for jp in range(4)]
            xst = [xsp.tile([128, 2, N], fp8, name=f"x{jp}")
                   for jp in range(4)]
            xsrc = [xT_d[jp * 128:(jp + 1) * 128, :].rearrange(
                "p (two f) -> p two f", two=2) for jp in range(4)]
            wqksrc = [wqk_d[jp * 128:(jp + 1) * 128, :].rearrange(
                "p (two f) -> p two f", two=2) for jp in range(4)]

            def load_x_quarter(q):
                for jp in range(4):
                    nc.sync.dma_start(
                        xst[jp][:, :, q * 1024:(q + 1) * 1024],
                        xsrc[jp][:, :, q * 1024:(q + 1) * 1024])

            for jp in range(4):
                nc.sync.dma_start(xst[jp][:, :, 0:256],
                                  xsrc[jp][:, :, 0:256])
            for jp in range(4):
                nc.sync.dma_start(wqk[jp][:, :, 0:512],
                                  wqksrc[jp][:, :, 0:512])
            for jp in range(4):
                nc.sync.dma_start(wqk[jp][:, :, 512:1024],
                                  wqksrc[jp][:, :, 512:1024])
            for jp in range(4):
                nc.sync.dma_start(xst[jp][:, :, 256:1024],
                                  xsrc[jp][:, :, 256:1024])
            load_x_quarter(1)
            wv = []
            for jp in range(4):
                w = wgtp.tile([128, 2, 512], fp8, name=f"wv{jp}")
                nc.sync.dma_start(
                    w[:], wv_d[jp * 128:(jp + 1) * 128, :].rearrange(
                        "p (two f) -> p two f", two=2))
                wv.append(w)
            load_x_quarter(2)
            load_x_quarter(3)
            wp = []
            for m in range(4):
                w = wgtp.tile([128, 2, 1024], fp8, name=f"wp{m}")
                nc.sync.dma_start(
                    w[:], wp_d[m * 128:(m + 1) * 128, :].rearrange(
                        "p (two f) -> p two f", two=2))
                wp.append(w)
            # prefetch the full residual during A1 (DMA is otherwise idle)
            xres = [xsp.tile([128, 2048], f32, name=f"xres{j}")
                    for j in range(8)]
            for j in range(8):
                nc.sync.dma_start(xres[j][:],
                                  xrT_d[j * 128:(j + 1) * 128, :])

            v_sb = [vop.tile([128, 2, N], fp8, name=f"v{hh}")
                    for hh in range(2)]
            # Gram accumulators (head-pair kq) + self-Gram diag blocks
            stG = [ps.tile([128, 512], f32, tag=f"pg{hh}", name=f"stG{hh}")
                   for hh in range(2)]
            qqd = ps.tile([128, 512], f32, tag="qqd", name="qqd")
            kkd = ps.tile([128, 512], f32, tag="kkd", name="kkd")

            # ---------------- A1: q,k + Grams + self-Gram diag ----------
            qcp = kcp = None
            for t in range(NT):
                pr = t // 2
                qp = ps.tile([128, 512], f32, tag=f"pq{t % 2}", name="qp")
                kp = ps.tile([128, 512], f32, tag=f"pk{t % 2}", name="kp")
                for jp in range(4):
                    lhs = xst[jp][:, :, t * 128:(t + 1) * 128]
                    nc.tensor.matmul(qp[:], lhs, wqk[jp][:, :, 0:512],
                                     start=(jp == 0), stop=(jp == 3),
                                     perf_mode=DR)
                    nc.tensor.matmul(kp[:], lhs, wqk[jp][:, :, 512:1024],
                                     start=(jp == 0), stop=(jp == 3),
                                     perf_mode=DR)
                if t % 2 == 0:
                    qcp = wrk.tile([128, 2, 512], fp8, tag="qcol", bufs=2,
                                   name="qcp")
                    kcp = wrk.tile([128, 2, 512], fp8, tag="kcol", bufs=2,
                                   name="kcp")
                nc.vector.tensor_copy(qcp[:, t % 2, :], qp[:])
                nc.scalar.copy(kcp[:, t % 2, :], kp[:])
                if t % 2 == 1:
                    fl, ll = (pr == 0), (pr == NT // 2 - 1)
                    for hh in range(2):
                        for m in range(2):
                            ks = kcp[:, :, hh * 256 + m * 128:
                                     hh * 256 + (m + 1) * 128]
                            qs = qcp[:, :, hh * 256 + m * 128:
                                     hh * 256 + (m + 1) * 128]
                            nc.tensor.matmul(
                                stG[hh][:, m * 256:(m + 1) * 256], ks,
                                qcp[:, :, hh * 256:(hh + 1) * 256],
                                start=fl, stop=ll, perf_mode=DR,
                                skip_group_check=True)
                            blk = slice((2 * hh + m) * 128,
                                        (2 * hh + m + 1) * 128)
                            nc.tensor.matmul(qqd[:, blk], qs, qs,
                                             start=fl, stop=ll, perf_mode=DR,
                                             skip_group_check=True)
                            nc.tensor.matmul(kkd[:, blk], ks, ks,
                                             start=fl, stop=ll, perf_mode=DR,
                                             skip_group_check=True)

            # ---------------- norms -> rqk [128, 8] ----------------
            # rqk col 0-3: temp/||q|| for (hh,m); col 4-7: 1/||k||
            dq = wrk.tile([128, 512], f32, tag="sth", bufs=2, name="dq")
            dk = wrk.tile([128, 512], f32, tag="sft", bufs=2, name="dk")
            nc.vector.tensor_mul(dq[:], qqd[:], identx[:])
            nc.vector.tensor_mul(dk[:], kkd[:], identx[:])
            rqk = constp.tile([128, 8], f32, name="rqk")
            nc.vector.reduce_sum(
                rqk[:, 0:4], dq[:].rearrange("p (c f) -> p c f", c=4), axis=AX)
            nc.vector.reduce_sum(
                rqk[:, 4:8], dk[:].rearrange("p (c f) -> p c f", c=4), axis=AX)
            nc.scalar.sqrt(rqk[:], rqk[:])
            nc.vector.tensor_scalar_max(rqk[:], rqk[:], EPS)
            nc.vector.reciprocal(rqk[:], rqk[:])
            nc.vector.tensor_mul(rqk[:, 0:4], rqk[:, 0:4], tmpv_sb[:])

            # ---------------- A2: v (overlaps B_h0 softmax prep) -------
            VVT = ["pq0", "pq1", "qqd", "kkd"]

            def vchunk(cv, sts, tags):
                for i, st in enumerate(sts):
                    vp = ps.tile([128, 512], f32, tag=tags[i % len(tags)],
                                 name="vp")
                    for jp in range(4):
                        nc.tensor.matmul(
                            vp[:], wv[jp][:, :, cv * 128:(cv + 1) * 128],
                            xst[jp][:, :, st * 512:(st + 1) * 512],
                            start=(jp == 0), stop=(jp == 3), perf_mode=DR)
                    dst = v_sb[cv // 2][:, cv % 2, st * 512:(st + 1) * 512]
                    if st % 2 == 0:
                        nc.vector.tensor_copy(dst, vp[:])
                    else:
                        nc.scalar.copy(dst, vp[:])

            for cv in range(4):
                vchunk(cv, list(range(8)), VVT)

            # ---------------- B + C per head ----------------
            # emission order: softmax(0), attnv(0), softmax(1), proj(0),
            # attnv(1), proj(1) -- so head 1's softmax (DVE/Act) overlaps
            # head 0's attn@v + proj (PE), keeping every engine streaming.
            OT = [[None] * 4, [None] * 4]
            ATN = [None, None]
            RCP = [None, None]

            def softmax(hh):
                sth = wrk.tile([128, 512], f32, tag="sth", bufs=2, name="sth")
                for m in range(2):
                    nc.vector.tensor_scalar_mul(
                        sth[:, m * 256:(m + 1) * 256],
                        stG[hh][:, m * 256:(m + 1) * 256],
                        rqk[:, 4 + 2 * hh + m:5 + 2 * hh + m])
                spm = ps.tile([128, 512], f32, tag="pk0", name="spm")
                for mc in range(2):
                    for md in range(2):
                        nc.tensor.transpose(
                            spm[:, mc * 256 + md * 128:
                                mc * 256 + (md + 1) * 128],
                            sth[:, md * 256 + mc * 128:
                                md * 256 + (mc + 1) * 128],
                            ident[:])
                sft = wrk.tile([128, 512], f32, tag="sft", bufs=2, name="sft")
                for mc in range(2):
                    nc.vector.tensor_scalar_mul(
                        sft[:, mc * 256:(mc + 1) * 256],
                        spm[:, mc * 256:(mc + 1) * 256],
                        rqk[:, 2 * hh + mc:1 + 2 * hh + mc])
                negmax = wrk.tile([128, 2], f32, tag="negmax", bufs=2,
                                  name="negmax")
                rowsum = wrk.tile([128, 2], f32, tag="rowsum", bufs=2,
                                  name="rowsum")
                recip = wrk.tile([128, 2], f32, tag="recip", bufs=2,
                                 name="recip")
                RCP[hh] = recip
                esb = wrk.tile([128, 512], f32, tag="esb", bufs=2, name="esb")
                for mc in range(2):
                    nc.vector.reduce_max(negmax[:, mc:mc + 1],
                                         sft[:, mc * 256:(mc + 1) * 256],
                                         axis=AX, negate=True)
                    nc.scalar.activation(esb[:, mc * 256:(mc + 1) * 256],
                                         sft[:, mc * 256:(mc + 1) * 256],
                                         Exp, bias=negmax[:, mc:mc + 1],
                                         accum_out=rowsum[:, mc:mc + 1])
                nc.vector.reciprocal(recip[:], rowsum[:])
                atp = ps.tile([128, 512], f32, tag="pk1", name="atp")
                for md in range(2):
                    for mc in range(2):
                        nc.tensor.transpose(
                            atp[:, md * 256 + mc * 128:
                                md * 256 + (mc + 1) * 128],
                            esb[:, mc * 256 + md * 128:
                                mc * 256 + (md + 1) * 128],
                            ident[:])
                atn = wrk.tile([128, 512], fp8, tag="atn", bufs=2,
                               name="atn")
                ATN[hh] = atn
                nc.vector.tensor_copy(atn[:], atp[:])

            def attnv(hh, ms, OVT):
                # attn @ v -> outT tiles [128, 2, 1024] fp8 (DR pairs = mc)
                atn, recip = ATN[hh], RCP[hh]
                pcnt = 0
                atn2 = atn[:].rearrange("p (two f) -> p two f", two=2)
                for m in ms:
                    ot = vop.tile([128, 2, 1024], fp8, tag=f"ot{m}", bufs=2,
                                  name=f"ot{hh}_{m}")
                    OT[hh][m] = ot
                    for mc in range(2):
                        for jh in range(2):
                            op = ps.tile([128, 512], f32,
                                         tag=OVT[pcnt % len(OVT)], name="op")
                            pcnt += 1
                            nc.tensor.matmul(
                                op[:], atn2[:, :, mc * 128:(mc + 1) * 128],
                                v_sb[hh][:, :, m * 1024 + jh * 512:
                                          m * 1024 + (jh + 1) * 512],
                                start=True, stop=True, perf_mode=DR)
                            dst = ot[:, mc, jh * 512:(jh + 1) * 512]
                            if mc == 0:
                                nc.vector.tensor_scalar_mul(
                                    dst, op[:], recip[:, mc:mc + 1])
                            else:
                                nc.scalar.activation(
                                    dst, op[:], Ident,
                                    scale=recip[:, mc:mc + 1])

            def proj(hh, js, poff=0):
                PJT = ["pq0", "pq1", "pk1", "pk0"]
                mult = mybir.AluOpType.mult
                add = mybir.AluOpType.add
                for j in js:
                    yq = xsp.tile([128, 1024], f32, tag="yq", bufs=3,
                                  name=f"yq{hh}_{j}")
                    for jh in range(2):
                        pidx = (poff + js.index(j) * 2 + jh) % len(PJT)
                        pp = ps.tile([128, 512], f32, tag=PJT[pidx],
                                     name="pp")
                        for m in range(4):
                            nc.tensor.matmul(
                                pp[:], wp[m][:, :, j * 128:(j + 1) * 128],
                                OT[hh][m][:, :, jh * 512:(jh + 1) * 512],
                                start=(m == 0), stop=(m == 3), perf_mode=DR)
                        dst = yq[:, jh * 512:(jh + 1) * 512]
                        if jh == 0:
                            nc.scalar.activation(dst, pp[:], Ident,
                                                 bias=bias_sb[:, j:j + 1],
                                                 scale=1.0 / (SW * SW))
                        else:
                            nc.vector.tensor_scalar(
                                dst, pp[:], 1.0 / (SW * SW),
                                bias_sb[:, j:j + 1], op0=mult, op1=add)
                    xrs = xres[j][:, hh * 1024:(hh + 1) * 1024]
                    if hh == 0 and j % 2 == 0:
                        nc.gpsimd.tensor_add(yq[:], yq[:], xrs)
                    else:
                        nc.vector.tensor_add(yq[:], yq[:], xrs)
                    nc.sync.dma_start(
                        yT_d[j * 128:(j + 1) * 128,
                             hh * 1024:(hh + 1) * 1024],
                        yq[:])

            softmax(0)
            softmax(1)
            attnv(0, range(4), ["kkd", "qqd", "pg0", "pk0"])
            for m in range(4):
                attnv(1, [m], ["kkd", "qqd", "pg1"])
                proj(0, [2 * m, 2 * m + 1], poff=m)
            proj(1, list(range(8)))

    nc.compile()
    return nc


def _get_nc():
    if "nc" not in _CACHE:
        _CACHE["nc"] = _build()
    return _CACHE["nc"]


def _drpack(a):
    """[1024, F] -> [512, 2F]: row 128j+p, free (i, f) = a[256j+128i+p, f]."""
    f = a.shape[1]
    return np.ascontiguousarray(
        a.reshape(4, 2, 128, f).transpose(0, 2, 1, 3).reshape(512, 2 * f))


def _make_in_maps(x, Wqkv, Wproj, bproj, temperature):
    import ml_dtypes

    fp8 = ml_dtypes.float8_e4m3
    x = np.ascontiguousarray(np.asarray(x, dtype=np.float32))
    Wqkv = np.asarray(Wqkv, dtype=np.float32)
    Wproj = np.asarray(Wproj, dtype=np.float32)
    bproj = np.asarray(bproj, dtype=np.float32).reshape(C)
    temp = np.asarray(temperature, dtype=np.float32).reshape(H)

    WqkvT = Wqkv.T  # [C, 3C]
    # token permutation: local l = m*1024 + j'  ->  global n = 4j' + m
    ell = np.arange(N)
    perm = 4 * (ell % 1024) + ell // 1024

    # per-batch fp8 x in paired-ktile layout
    xdr = []
    for b in range(B):
        xp = np.ascontiguousarray(x[b].T[:, perm])
        xdr.append(_drpack(xp).astype(fp8))

    # per-head-pair weights
    wqk_hp, wv_hp = [], []
    for hp in range(2):
        h0, h1 = 2 * hp, 2 * hp + 1
        qk = np.concatenate([
            WqkvT[:, 256 * h0:256 * h0 + 256],
            WqkvT[:, 256 * h1:256 * h1 + 256],
            WqkvT[:, 1024 + 256 * h0:1024 + 256 * h0 + 256],
            WqkvT[:, 1024 + 256 * h1:1024 + 256 * h1 + 256]], axis=1)
        vv = np.concatenate([
            WqkvT[:, 2048 + 256 * h0:2048 + 256 * h0 + 256],
            WqkvT[:, 2048 + 256 * h1:2048 + 256 * h1 + 256]], axis=1)
        wqk_hp.append(_drpack(qk * SW).astype(fp8))
        wv_hp.append(_drpack(vv * SW).astype(fp8))
    wp8 = _drpack(Wproj.T * SW).astype(fp8)
    bias2d = np.ascontiguousarray(bproj.reshape(8, 128).T)

    in_maps = []
    for core in range(NCORES):
        b, hp = core // 2, core % 2
        h0, h1 = 2 * hp, 2 * hp + 1
        xrT = np.ascontiguousarray(
            x[b, 2048 * hp:2048 * hp + 2048, :].T)
        tmpv = np.broadcast_to(
            np.array([temp[h0], temp[h0], temp[h1], temp[h1]],
                     dtype=np.float32), (128, 4)).copy()
        in_maps.append(dict(xT=xdr[b], wqk=wqk_hp[hp], wv=wv_hp[hp],
                            wp=wp8, xrT=xrT, bias=bias2d, tmpv=tmpv))
    return in_maps


def _run(in_maps, trace=False, **kw):
    from concourse.bass_utils import run_bass_kernel_spmd

    nc = _get_nc()
    return run_bass_kernel_spmd(nc, in_maps, core_ids=list(range(NCORES)),
                                trace=trace, **kw)


def kernel(x, Wqkv, Wproj, bproj, temperature):
    res = _run(_make_in_maps(x, Wqkv, Wproj, bproj, temperature))
    y = np.empty((B, N, C), dtype=np.float32)
    for core in range(NCORES):
        b, hp = core // 2, core % 2
        y[b, 2048 * hp:2048 * hp + 2048, :] = res.results[core]["yT"].T
    return y
